# revision 17
# baseline (speedup 1.0000x reference)
"""GAT layer kernel for Trainium2, sharded over 8 NeuronCores.

Math (per batch b):
    h   = x @ W                      [N, F]
    s1  = h @ a1, s2 = h @ a2        [N]
    e   = leaky_relu(s1[:,None] + s2[None,:], 0.2)
    att = softmax(where(adj>0, e, -9e15), axis=-1)
    out = elu(att @ h)

Sharding: destination rows (i) are split 512/core.  Each core gets a
row-rolled copy of x^T so that "my rows" are always rows 0..512 of the
rolled ordering (keeps the SPMD program identical across cores).  Each
core computes h for ALL (rolled) source rows redundantly, then does the
masked-softmax attention for its own 512 destination rows.

On-chip layout for the attention stage: partition = j (source node,
128/chunk, 32 chunks), free = i (my 512 destination rows).
    lrelu:  ACT activation(Lrelu, in=s1_bcast[128,512], bias=s2[j], alpha=0.2)
    exp:    ACT activation(Exp) over 4-chunk groups, bf16 out
    mask:   DVE tensor_tensor mult with bf16 adj mask
    P@[h|1]: PE matmuls accumulating [i,65] (65th col = rowsum)
    post:   1/rowsum scale + ELU composed from min/exp/max/stt
"""

import numpy as np

BS, N, F_IN, F_OUT = 4, 4096, 512, 64
NCORES = 8
ROWS = N // NCORES          # 512 destination rows per core
NCH = N // 128              # 32 source chunks
KC = F_IN // 128            # 4 contraction chunks
MSUB = N // 128             # 32 row subtiles for the h matmul
ALPHA = 0.2

_CACHE = {}


def _build_nc(dbg=False):
    import concourse.bacc as bacc
    import concourse.mybir as mybir
    from concourse import tile

    f32 = mybir.dt.float32
    bf16 = mybir.dt.bfloat16
    i32 = mybir.dt.int32
    AF = mybir.ActivationFunctionType
    OP = mybir.AluOpType

    nc = bacc.Bacc(
        "TRN2",
        target_bir_lowering=False,
        debug=False,
        enable_asserts=False,
        num_devices=NCORES,
    )

    xT = nc.dram_tensor("xT", [BS, F_IN, N], f32, kind="ExternalInput")
    adjT = nc.dram_tensor("adjT", [N, ROWS], i32, kind="ExternalInput")
    W = nc.dram_tensor("W", [F_IN, F_OUT], f32, kind="ExternalInput")
    WT = nc.dram_tensor("WT", [F_OUT, F_IN], f32, kind="ExternalInput")
    a_in = nc.dram_tensor("a", [2 * F_OUT, 1], f32, kind="ExternalInput")
    out = nc.dram_tensor("out", [BS, ROWS, F_OUT], f32, kind="ExternalOutput")
    if dbg:
        hbf_dbg = nc.dram_tensor("hbf_dbg", [128, NCH * 65], bf16, kind="ExternalOutput")
        s12_dbg = nc.dram_tensor("s12_dbg", [128, 2 * MSUB], f32, kind="ExternalOutput")
        s1bc_dbg = nc.dram_tensor("s1bc_dbg", [128, ROWS], f32, kind="ExternalOutput")
        pm_dbg = nc.dram_tensor("pm_dbg", [4, 128, ROWS], bf16, kind="ExternalOutput")
        lb_dbg = nc.dram_tensor("lb_dbg", [128, 4 * ROWS], f32, kind="ExternalOutput")
        rs_dbg = nc.dram_tensor("rs_dbg", [128, 4], f32, kind="ExternalOutput")

    with tile.TileContext(nc) as tc:
        with (
            tc.tile_pool(name="const", bufs=1) as cpool,
            tc.tile_pool(name="mask", bufs=1) as mpool,
            tc.tile_pool(name="adjst", bufs=4) as apool,
            tc.tile_pool(name="xin", bufs=10) as xpool,
            tc.tile_pool(name="work", bufs=2) as wpool,
            tc.tile_pool(name="pm", bufs=4) as pmpool,
            tc.tile_pool(name="psum", bufs=1, space="PSUM") as ppool,
            tc.tile_pool(name="pso", bufs=1, space="PSUM") as opool,
            tc.tile_pool(name="dram", bufs=2, space="DRAM") as dpool,
        ):
            # ---------------- constants / prep ----------------
            # W_aug chunks: [128, 66] per kc -> cols 0:64 = W chunk, 64:66 = [w1|w2]
            waug = cpool.tile([128, KC * 66], f32)
            for kc in range(KC):
                nc.sync.dma_start(
                    waug[:, kc * 66 : kc * 66 + 64],
                    W[kc * 128 : (kc + 1) * 128, :],
                )
            wtt = cpool.tile([F_OUT, F_IN], f32)
            nc.sync.dma_start(wtt[:, :], WT[:, :])
            a12 = cpool.tile([F_OUT, 2], f32)
            nc.sync.dma_start(a12[:, 0:1], a_in[0:F_OUT, :])
            nc.sync.dma_start(a12[:, 1:2], a_in[F_OUT:, :])
            # w12T[kc] = WT[:, kc].T @ a12  -> [128, 2]
            for kc in range(KC):
                psw = ppool.tile([128, 66], f32, tag="psa0", name="psw")
                nc.tensor.matmul(
                    psw[:, 0:2],
                    wtt[:, kc * 128 : (kc + 1) * 128],
                    a12[:, :],
                    start=True,
                    stop=True,
                )
                nc.vector.tensor_copy(waug[:, kc * 66 + 64 : kc * 66 + 66], psw[:, 0:2])

            ones_bf = cpool.tile([128, 1], bf16)
            nc.vector.memset(ones_bf[:, :], 1.0)
            alpha02 = cpool.tile([128, 1], f32)
            nc.vector.memset(alpha02[:, :], ALPHA)

            # bf16 mask, [j, i] layout: chunk ch lives at cols ch*512..
            maskf = mpool.tile([128, NCH * ROWS], bf16)
            for ch in range(NCH):
                adjst = apool.tile([128, ROWS], i32, tag="adjst")
                nc.sync.dma_start(adjst[:, :], adjT[ch * 128 : (ch + 1) * 128, :])
                nc.vector.tensor_copy(
                    maskf[:, ch * ROWS : (ch + 1) * ROWS], adjst[:, :]
                )

            # ---------------- per-batch pipeline ----------------
            for b in range(BS):
                # ---- h = x @ W_aug for ALL rolled rows (redundant) ----
                # hbf[j-part, ch*65 : ch*65+64] = h chunk (bf16), col +64 = 1.0
                hbf = wpool.tile([128, NCH * 65], bf16, tag="hbf")
                nc.vector.memset(hbf[:, :].rearrange("p (c o) -> p c o", o=65)[:, :, 64:65], 1.0)
                s12 = wpool.tile([128, 2 * MSUB], f32, tag="s12")
                for g in range(8):  # groups of 4 msubs
                    # one PSUM bank per accumulation group: start=True
                    # clears the whole bank's has_written state, so
                    # concurrent groups must not share a bank
                    psam = [
                        ppool.tile([128, 66], f32, tag=f"psa{m}", name=f"psa{m}") for m in range(4)
                    ]
                    for kc in range(KC):
                        xg = xpool.tile([128, 512], f32, tag="xg")
                        nc.sync.dma_start(
                            xg[:, :],
                            xT[b, kc * 128 : (kc + 1) * 128, g * 512 : (g + 1) * 512],
                        )
                        for m in range(4):
                            nc.tensor.matmul(
                                psam[m][:, :],
                                xg[:, m * 128 : (m + 1) * 128],
                                waug[:, kc * 66 : (kc + 1) * 66],
                                start=(kc == 0),
                                stop=(kc == KC - 1),
                            )
                    # evacuate: h part (fp32->bf16), s12 part (fp32)
                    for m in range(4):
                        ms = g * 4 + m
                        nc.vector.tensor_copy(
                            hbf[:, ms * 65 : ms * 65 + 64], psam[m][:, 0:64]
                        )
                        nc.vector.tensor_copy(
                            s12[:, ms * 2 : ms * 2 + 2], psam[m][:, 64:66]
                        )

                # ---- s1 broadcast tile: my rows are rolled rows 0..512 ----
                s1row = dpool.tile([ROWS], f32, tag="s1row")
                for m in range(4):
                    nc.sync.dma_start(
                        s1row[m * 128 : (m + 1) * 128], s12[:, 2 * m : 2 * m + 1]
                    )
                s1bc = wpool.tile([128, ROWS], f32, tag="s1bc")
                nc.sync.dma_start(s1bc[0:1, :], s1row[:].rearrange("(p r) -> p r", p=1))
                k = 1
                while k < 128:
                    nc.sync.dma_start(s1bc[k : 2 * k, :], s1bc[0:k, :])
                    k *= 2

                if dbg and b == 0:
                    nc.sync.dma_start(hbf_dbg[:, :], hbf[:, :])
                    nc.sync.dma_start(s12_dbg[:, :], s12[:, :])
                    nc.sync.dma_start(s1bc_dbg[:, :], s1bc[:, :])

                # ---- attention ----
                psom = [opool.tile([128, 65], f32, tag=f"pso{i}", name=f"pso{i}") for i in range(4)]
                for cg in range(8):  # 4 chunks per exp group
                    lbuf = wpool.tile([128, 4 * ROWS], f32, tag="lbuf")
                    for q in range(4):
                        ch = cg * 4 + q
                        nc.scalar.activation(
                            lbuf[:, q * ROWS : (q + 1) * ROWS],
                            s1bc[:, :],
                            AF.Prelu,
                            bias=s12[:, 2 * ch + 1 : 2 * ch + 2],
                            scale=1.0,
                            alpha=alpha02[:, :],
                        )
                    if dbg and b == 0 and cg == 0:
                        nc.sync.dma_start(lb_dbg[:, :], lbuf[:, :])
                    pbuf = wpool.tile([128, 4 * ROWS], bf16, tag="pbuf")
                    nc.scalar.activation(pbuf[:, :], lbuf[:, :], AF.Exp)
                    for q in range(4):
                        ch = cg * 4 + q
                        pm = pmpool.tile([128, ROWS], bf16, tag="pm")
                        nc.vector.tensor_tensor(
                            pm[:, :],
                            pbuf[:, q * ROWS : (q + 1) * ROWS],
                            maskf[:, ch * ROWS : (ch + 1) * ROWS],
                            OP.mult,
                        )
                        if dbg and b == 0 and cg == 0:
                            nc.sync.dma_start(pm_dbg[q, :, :], pm[:, :])
                        for isub in range(4):
                            nc.tensor.matmul(
                                psom[isub][:, :],
                                pm[:, isub * 128 : (isub + 1) * 128],
                                hbf[:, ch * 65 : (ch + 1) * 65],
                                start=(ch == 0),
                                stop=(ch == NCH - 1),
                            )

                # ---- normalize + ELU + store ----
                rs = wpool.tile([128, 4], f32, tag="rs")
                for isub in range(4):
                    nc.vector.tensor_copy(
                        rs[:, isub : isub + 1], psom[isub][:, 64:65]
                    )
                if dbg and b == 0:
                    nc.sync.dma_start(rs_dbg[:, :], rs[:, :])
                rcp = wpool.tile([128, 4], f32, tag="rcp")
                nc.vector.reciprocal(rcp[:, :], rs[:, :])
                hp = wpool.tile([128, 4 * 64], f32, tag="hp")
                for isub in range(4):
                    nc.vector.tensor_scalar_mul(
                        hp[:, isub * 64 : (isub + 1) * 64],
                        psom[isub][:, 0:64],
                        rcp[:, isub : isub + 1],
                    )
                npart = wpool.tile([128, 4 * 64], f32, tag="npart")
                nc.vector.tensor_scalar_min(npart[:, :], hp[:, :], 0.0)
                expn = wpool.tile([128, 4 * 64], f32, tag="expn")
                nc.scalar.activation(expn[:, :], npart[:, :], AF.Exp)
                outf = wpool.tile([128, 4 * 64], f32, tag="outf")
                # outf = (expn - 1) + relu(hp);  relu via max into npart reuse
                relu_t = wpool.tile([128, 4 * 64], f32, tag="relu_t")
                nc.vector.tensor_scalar_max(relu_t[:, :], hp[:, :], 0.0)
                nc.vector.scalar_tensor_tensor(
                    outf[:, :], expn[:, :], -1.0, relu_t[:, :], OP.add, OP.add
                )
                for isub in range(4):
                    nc.sync.dma_start(
                        out[b, isub * 128 : (isub + 1) * 128, :],
                        outf[:, isub * 64 : (isub + 1) * 64],
                    )
    return nc


def _get_nc():
    if "nc" not in _CACHE:
        nc = _build_nc()
        nc.finalize()
        _CACHE["nc"] = nc
    return _CACHE["nc"]


def _install_ntff_hook():
    """The agent image's antenv lacks axon_hooks; recreate it so
    run_bass_kernel_spmd(trace=True) can reach the NTFF profiler."""
    import sys, types

    if "antenv.axon_hooks" in sys.modules:
        return
    try:
        from trn_agent_boot.trn_boot import _ntff_profile_via_ctypes
    except ImportError:
        return
    hook = _ntff_profile_via_ctypes("/opt/axon/libaxon_pjrt.so")
    mod = types.ModuleType("antenv.axon_hooks")
    mod._hook = hook
    mod.set_axon_ntff_profile_hook = lambda h: setattr(mod, "_hook", h)
    mod.get_axon_ntff_profile_hook = lambda: mod._hook
    sys.modules["antenv.axon_hooks"] = mod


def kernel(x, adj, W, a, _trace=False):
    from concourse.bass_utils import run_bass_kernel_spmd

    if _trace:
        _install_ntff_hook()

    x = np.asarray(x)
    adj = np.asarray(adj)
    W = np.asarray(W)
    a = np.asarray(a)

    xT = np.ascontiguousarray(x.transpose(0, 2, 1))  # [BS, F_IN, N]
    WT = np.ascontiguousarray(W.T)

    in_maps = []
    for c in range(NCORES):
        c0 = c * ROWS
        # roll so this core's rows are rolled-rows 0..ROWS
        xT_c = np.ascontiguousarray(np.roll(xT, -c0, axis=2))
        adjT_c = np.ascontiguousarray(
            np.roll(adj[c0 : c0 + ROWS, :], -c0, axis=1).T
        )  # [N(j'), ROWS(i)]
        in_maps.append({"xT": xT_c, "adjT": adjT_c, "W": W, "WT": WT, "a": a})

    nc = _get_nc()
    res = run_bass_kernel_spmd(
        nc, in_maps, core_ids=list(range(NCORES)), trace=_trace
    )
    outs = [r["out"] for r in res.results]
    full = np.concatenate(outs, axis=1)  # [BS, N, F_OUT]
    if _trace:
        return full, res
    return full
